# revision 14
# baseline (speedup 1.0000x reference)
"""BOW multi-hot regression kernel for trn2, 8 NeuronCores.

score[b, l] = sum_{v in distinct non-PAD tokens of doc b} W[l, v] + bias[l]

Strategy (V-sharded, single SPMD launch):
  - vocab padded to 50176 = 8 * 6272; core g owns rows [6272g, 6272(g+1)).
  - W.T is host-prepped to bf16, PAD column zeroed, bias appended as row 50175
    which is activated through two constant extra token slots per doc.
  - per core: DVE computes per-chunk masked int16 indices; GPSIMD local_scatter
    builds the multi-hot bow [128 docs, 6272] bf16 per doc-tile (duplicate
    tokens overwrite the same cell with the same 1.0 -> dedup for free);
    HWDGE xbar DMA transposes it to ktile layout [128 v, 49, 128 docs];
    PE accumulates 49 matmuls per doc-tile into PSUM -> partial [1024, 512];
    one bf16 ReduceScatter sums partials; core g outputs docs [128g, 128(g+1)).
"""

import sys

sys.path.insert(0, "/opt/trn_rl_repo")

import numpy as np
import ml_dtypes

from concourse import bass, bacc, tile, mybir, bass_utils
from concourse.tile import add_dep_helper

# problem constants
T, B, V, L = 200, 1024, 50000, 512
PAD = 1
NCORES = 8
VP = 50176            # padded vocab, = NCORES * VC
VC = VP // NCORES     # 6272 vocab rows per core = KT * 128
KT = VC // 128        # 49 ktiles
CH = 1568             # local_scatter chunk width (1568 * 32 < 2**16)
NCH = VC // CH        # 4 chunks
TS = T + 2            # token slots per doc: 200 real + bias slot + filler
DT = B // 128         # 8 doc-tiles
BIAS_SLOT = VP - 1    # 50175
FILL_SLOT = VP - 2    # 50174
TOKW = DT * TS + NCH * TS  # flat tok row: DT doc slices then NCH chunk bases
RS_SPLIT = (6, 2)         # doc-tiles per ReduceScatter chunk

_cache = {}


def _build_nc():
    nc = bacc.Bacc("TRN2", target_bir_lowering=False, debug=False,
                   num_devices=NCORES)
    f32 = mybir.dt.float32
    bf16 = mybir.dt.bfloat16
    i32 = mybir.dt.int32
    i16 = mybir.dt.int16
    Alu = mybir.AluOpType

    # tok row layout per partition p: [DT, TS] tokens of docs {dt*128+p},
    # then [NCH, TS] chunk bases (core-local, host-filled)
    tok_d = nc.dram_tensor("tok", [128, TOKW], i32, kind="ExternalInput")
    wt_d = nc.dram_tensor("wt", [128, KT, 512], bf16, kind="ExternalInput")
    out_d = nc.dram_tensor("out", [128, 512], f32, kind="ExternalOutput")

    W808 = NCH * TS  # 808

    with tile.TileContext(nc) as tc:
        with tc.tile_pool(name="const", bufs=1) as cpool, \
             tc.tile_pool(name="work", bufs=3) as wpool, \
             tc.tile_pool(name="bow", bufs=3) as bpool, \
             tc.tile_pool(name="bowt", bufs=3) as btpool, \
             tc.tile_pool(name="psum", bufs=8, space="PSUM") as ppool, \
             tc.tile_pool(name="dram", bufs=1, space="DRAM") as dpool:

            tok_sb = cpool.tile([128, TOKW], i32, tag="tok")
            tok_dma = nc.sync.dma_start(out=tok_sb[:], in_=tok_d.ap())

            # wt: first ktiles immediately (needed by the first matmuls),
            # bulk deferred behind tok so tok wins the HBM bandwidth race
            wt_sb = cpool.tile([128, KT, 512], bf16, tag="wt")
            nc.scalar.dma_start(out=wt_sb[:, :8, :], in_=wt_d.ap()[:, :8, :])
            wt_bulk = nc.scalar.dma_start(
                out=wt_sb[:, 8:, :], in_=wt_d.ap()[:, 8:, :]
            )
            add_dep_helper(wt_bulk.ins, tok_dma.ins, sync=True,
                           reason="tok DMA gates the whole pipeline")

            ones_sb = cpool.tile([128, TS], bf16, tag="ones")
            nc.vector.memset(ones_sb[:], 1.0)

            # dummy scatter: loads the Q7 local_scatter library (~6us IRAM
            # DMA) during the preamble instead of on the critical path
            negi = cpool.tile([128, 2], i16, tag="negi")
            nc.vector.memset(negi[:], -1)
            scr = cpool.tile([128, 2], bf16, tag="scr")
            nc.gpsimd.local_scatter(
                scr[:], scr[:], negi[:], channels=128, num_elems=2, num_idxs=2,
            )

            partial_sb = cpool.tile([128, DT, 512], bf16, tag="partial")
            bases = tok_sb[:, DT * TS:].rearrange("p (c w) -> p c w", c=NCH)

            for dt in range(DT):
                # masked local chunk indices: for chunk c,
                # idx = tok - core_base - 1568*c  if in [0, 1568) else negative
                tokrep = (
                    tok_sb[:, dt * TS:(dt + 1) * TS]
                    .unsqueeze(1)
                    .broadcast_to((128, NCH, TS))
                )
                d_t = wpool.tile([128, NCH, TS], i32, tag="d")
                nc.vector.tensor_tensor(
                    out=d_t[:], in0=tokrep, in1=bases, op=Alu.subtract,
                )
                nc.vector.tensor_scalar(
                    out=d_t[:], in0=d_t[:],
                    scalar1=32767, scalar2=-1, op0=Alu.min, op1=Alu.max,
                )
                m_t = wpool.tile([128, NCH, TS], i32, tag="m")
                nc.vector.tensor_scalar(
                    out=m_t[:], in0=d_t[:],
                    scalar1=CH, scalar2=-32768, op0=Alu.is_ge, op1=Alu.mult,
                )
                idx_t = wpool.tile([128, NCH, TS], i16, tag="idx")
                nc.vector.tensor_tensor(
                    out=idx_t[:], in0=d_t[:], in1=m_t[:], op=Alu.add,
                )

                bow_t = bpool.tile([128, VC], bf16, tag="bow")
                for c in range(NCH):
                    nc.gpsimd.local_scatter(
                        bow_t[:, c * CH:(c + 1) * CH],
                        ones_sb[:],
                        idx_t[:, c, :],
                        channels=128,
                        num_elems=CH,
                        num_idxs=TS,
                    )

                bowt_t = btpool.tile([128, KT, 128], bf16, tag="bowt")
                ps = ppool.tile([128, 512], f32, tag="ps")
                k0 = 0
                for c in range(NCH):
                    k1 = ((c + 1) * CH) // 128 if c < NCH - 1 else KT
                    nc.sync.dma_start(
                        out=bowt_t[:, k0:k1, :],
                        in_=bow_t[:, k0 * 128:k1 * 128],
                        transpose=True,
                    )
                    for k in range(k0, k1):
                        nc.tensor.matmul(
                            out=ps[:],
                            lhsT=bowt_t[:, k, :],
                            rhs=wt_sb[:, k, :],
                            start=(k == 0),
                            stop=(k == KT - 1),
                        )
                    k0 = k1
                nc.vector.tensor_copy(out=partial_sb[:, dt, :], in_=ps[:])

            # chunked bf16 ReduceScatter: first chunk (dt 0..5) overlaps
            # the trailing matmuls; second chunk (dt 6..7) is the only
            # serial tail. Core g receives docs [96g, 96g+96) from chunk 0
            # and [768+32g, 768+32g+32) from chunk 1 (host reassembles).
            rs_tiles = []
            off = 0
            for h, ndt in enumerate(RS_SPLIT):
                pd = dpool.tile([ndt * 128, 512], bf16, tag=f"pdram{h}")
                nc.scalar.dma_start(
                    out=pd[:].rearrange("(d p) l -> p d l", p=128),
                    in_=partial_sb[:, off:off + ndt, :],
                )
                rs = dpool.tile([ndt * 16, 512], bf16, tag=f"rsout{h}")
                nc.gpsimd.collective_compute(
                    "ReduceScatter",
                    mybir.AluOpType.add,
                    replica_groups=[list(range(NCORES))],
                    ins=[pd.opt()],
                    outs=[rs.opt()],
                )
                rs_tiles.append(rs)
                off += ndt
            # SWDGE cast DMAs bf16 -> fp32 into the output
            r0 = 0
            for h, rs in enumerate(rs_tiles):
                n = RS_SPLIT[h] * 16
                nc.gpsimd.dma_start(
                    out=out_d.ap()[r0:r0 + n, :], in_=rs[:]
                )
                r0 += n

    nc.compile()
    return nc


def _host_prep(text, W, b):
    # tokens: [T, B] -> [B, T] int32, append bias + filler slots
    tok = np.ascontiguousarray(text.T).astype(np.int32)          # [B, T]
    extra = np.empty((B, 2), np.int32)
    extra[:, 0] = BIAS_SLOT
    extra[:, 1] = FILL_SLOT
    tok = np.concatenate([tok, extra], axis=1)                   # [B, TS]
    # partition-major pack: row p = docs {dt*128+p for dt in range(DT)}
    tok_pm = np.ascontiguousarray(
        tok.reshape(DT, 128, TS).transpose(1, 0, 2)
    ).reshape(128, DT * TS)

    chunk_base = np.repeat(np.arange(NCH, dtype=np.int32) * CH, TS)  # [NCH*TS]

    # weights: Wt [VP, 512] bf16, PAD column zeroed, bias row appended
    Wt = np.zeros((VP, L), np.float32)
    Wt[:V] = W.T
    Wt[PAD] = 0.0
    Wt[BIAS_SLOT] = b
    Wt = Wt.astype(ml_dtypes.bfloat16)

    in_maps = []
    for g in range(NCORES):
        bases_row = chunk_base + np.int32(g * VC)
        bases = np.broadcast_to(bases_row, (128, NCH * TS))
        tok_g = np.concatenate([tok_pm, bases], axis=1)          # [128, TOKW]
        wt_g = np.ascontiguousarray(
            Wt[g * VC:(g + 1) * VC].reshape(KT, 128, L).transpose(1, 0, 2)
        )                                                        # [128, KT, 512]
        in_maps.append({"tok": tok_g, "wt": wt_g})
    return in_maps


def kernel(text, W, b, trace=False, trace_kwargs=None):
    if "nc" not in _cache:
        _cache["nc"] = _build_nc()
    nc = _cache["nc"]
    in_maps = _host_prep(np.asarray(text), np.asarray(W), np.asarray(b))
    res = bass_utils.run_bass_kernel_spmd(
        nc, in_maps, core_ids=list(range(NCORES)),
        trace=trace, **(trace_kwargs or {}),
    )
    _cache["last_results"] = res
    out = np.empty((B, L), np.float32)
    for g in range(NCORES):
        og = res.results[g]["out"]
        r0, d0 = 0, 0
        for ndt in RS_SPLIT:
            n = ndt * 16
            out[d0 + n * g:d0 + n * g + n] = og[r0:r0 + n]
            r0 += n
            d0 += ndt * 128
    return out


# revision 15
# speedup vs baseline: 1.0245x; 1.0245x over previous
"""BOW multi-hot regression kernel for trn2, 8 NeuronCores.

score[b, l] = sum_{v in distinct non-PAD tokens of doc b} W[l, v] + bias[l]

Strategy (V-sharded, single SPMD launch):
  - vocab padded to 50176 = 8 * 6272; core g owns rows [6272g, 6272(g+1)).
  - W.T is host-prepped to bf16, PAD column zeroed, bias appended as row 50175
    which is activated through two constant extra token slots per doc.
  - per core: DVE computes per-chunk masked int16 indices; GPSIMD local_scatter
    builds the multi-hot bow [128 docs, 6272] bf16 per doc-tile (duplicate
    tokens overwrite the same cell with the same 1.0 -> dedup for free);
    HWDGE xbar DMA transposes it to ktile layout [128 v, 49, 128 docs];
    PE accumulates 49 matmuls per doc-tile into PSUM -> partial [1024, 512];
    one bf16 ReduceScatter sums partials; core g outputs docs [128g, 128(g+1)).
"""

import sys

sys.path.insert(0, "/opt/trn_rl_repo")

import numpy as np
import ml_dtypes

from concourse import bass, bacc, tile, mybir, bass_utils
from concourse.tile import add_dep_helper

# problem constants
T, B, V, L = 200, 1024, 50000, 512
PAD = 1
NCORES = 8
VP = 50176            # padded vocab, = NCORES * VC
VC = VP // NCORES     # 6272 vocab rows per core = KT * 128
KT = VC // 128        # 49 ktiles
CH = 1568             # local_scatter chunk width (1568 * 32 < 2**16)
NCH = VC // CH        # 4 chunks
TS = T + 2            # token slots per doc: 200 real + bias slot + filler
DT = B // 128         # 8 doc-tiles
BIAS_SLOT = VP - 1    # 50175
FILL_SLOT = VP - 2    # 50174
TOKW = DT * TS + NCH * TS  # flat tok row: DT doc slices then NCH chunk bases
RS_SPLIT = (1, 5, 2)       # doc-tiles per ReduceScatter chunk (first absorbs setup)

_cache = {}


def _build_nc():
    nc = bacc.Bacc("TRN2", target_bir_lowering=False, debug=False,
                   num_devices=NCORES)
    f32 = mybir.dt.float32
    bf16 = mybir.dt.bfloat16
    i32 = mybir.dt.int32
    i16 = mybir.dt.int16
    Alu = mybir.AluOpType

    # tok row layout per partition p: [DT, TS] tokens of docs {dt*128+p},
    # then [NCH, TS] chunk bases (core-local, host-filled)
    tok_d = nc.dram_tensor("tok", [128, TOKW], i32, kind="ExternalInput")
    wt_d = nc.dram_tensor("wt", [128, KT, 512], bf16, kind="ExternalInput")
    out_d = nc.dram_tensor("out", [128, 512], f32, kind="ExternalOutput")

    W808 = NCH * TS  # 808

    with tile.TileContext(nc) as tc:
        with tc.tile_pool(name="const", bufs=1) as cpool, \
             tc.tile_pool(name="work", bufs=3) as wpool, \
             tc.tile_pool(name="bow", bufs=3) as bpool, \
             tc.tile_pool(name="bowt", bufs=3) as btpool, \
             tc.tile_pool(name="psum", bufs=8, space="PSUM") as ppool, \
             tc.tile_pool(name="dram", bufs=1, space="DRAM") as dpool:

            tok_sb = cpool.tile([128, TOKW], i32, tag="tok")
            tok_dma = nc.sync.dma_start(out=tok_sb[:], in_=tok_d.ap())

            # wt: first ktiles immediately (needed by the first matmuls),
            # bulk deferred behind tok so tok wins the HBM bandwidth race
            wt_sb = cpool.tile([128, KT, 512], bf16, tag="wt")
            nc.scalar.dma_start(out=wt_sb[:, :8, :], in_=wt_d.ap()[:, :8, :])
            wt_bulk = nc.scalar.dma_start(
                out=wt_sb[:, 8:, :], in_=wt_d.ap()[:, 8:, :]
            )
            add_dep_helper(wt_bulk.ins, tok_dma.ins, sync=True,
                           reason="tok DMA gates the whole pipeline")

            ones_sb = cpool.tile([128, TS], bf16, tag="ones")
            nc.vector.memset(ones_sb[:], 1.0)

            # dummy scatter: loads the Q7 local_scatter library (~6us IRAM
            # DMA) during the preamble instead of on the critical path
            negi = cpool.tile([128, 2], i16, tag="negi")
            nc.vector.memset(negi[:], -1)
            scr = cpool.tile([128, 2], bf16, tag="scr")
            nc.gpsimd.local_scatter(
                scr[:], scr[:], negi[:], channels=128, num_elems=2, num_idxs=2,
            )

            partial_sb = cpool.tile([128, DT, 512], bf16, tag="partial")
            bases = tok_sb[:, DT * TS:].rearrange("p (c w) -> p c w", c=NCH)

            for dt in range(DT):
                # masked local chunk indices: for chunk c,
                # idx = tok - core_base - 1568*c  if in [0, 1568) else negative
                tokrep = (
                    tok_sb[:, dt * TS:(dt + 1) * TS]
                    .unsqueeze(1)
                    .broadcast_to((128, NCH, TS))
                )
                d_t = wpool.tile([128, NCH, TS], i32, tag="d")
                nc.vector.tensor_tensor(
                    out=d_t[:], in0=tokrep, in1=bases, op=Alu.subtract,
                )
                nc.vector.tensor_scalar(
                    out=d_t[:], in0=d_t[:],
                    scalar1=32767, scalar2=-1, op0=Alu.min, op1=Alu.max,
                )
                m_t = wpool.tile([128, NCH, TS], i32, tag="m")
                nc.vector.tensor_scalar(
                    out=m_t[:], in0=d_t[:],
                    scalar1=CH, scalar2=-32768, op0=Alu.is_ge, op1=Alu.mult,
                )
                idx_t = wpool.tile([128, NCH, TS], i16, tag="idx")
                nc.vector.tensor_tensor(
                    out=idx_t[:], in0=d_t[:], in1=m_t[:], op=Alu.add,
                )

                bow_t = bpool.tile([128, VC], bf16, tag="bow")
                for c in range(NCH):
                    nc.gpsimd.local_scatter(
                        bow_t[:, c * CH:(c + 1) * CH],
                        ones_sb[:],
                        idx_t[:, c, :],
                        channels=128,
                        num_elems=CH,
                        num_idxs=TS,
                    )

                bowt_t = btpool.tile([128, KT, 128], bf16, tag="bowt")
                ps = ppool.tile([128, 512], f32, tag="ps")
                k0 = 0
                for c in range(NCH):
                    k1 = ((c + 1) * CH) // 128 if c < NCH - 1 else KT
                    nc.sync.dma_start(
                        out=bowt_t[:, k0:k1, :],
                        in_=bow_t[:, k0 * 128:k1 * 128],
                        transpose=True,
                    )
                    for k in range(k0, k1):
                        nc.tensor.matmul(
                            out=ps[:],
                            lhsT=bowt_t[:, k, :],
                            rhs=wt_sb[:, k, :],
                            start=(k == 0),
                            stop=(k == KT - 1),
                        )
                    k0 = k1
                nc.vector.tensor_copy(out=partial_sb[:, dt, :], in_=ps[:])

            # chunked bf16 ReduceScatter: first chunk (dt 0..5) overlaps
            # the trailing matmuls; second chunk (dt 6..7) is the only
            # serial tail. Core g receives docs [96g, 96g+96) from chunk 0
            # and [768+32g, 768+32g+32) from chunk 1 (host reassembles).
            rs_tiles = []
            off = 0
            for h, ndt in enumerate(RS_SPLIT):
                pd = dpool.tile([ndt * 128, 512], bf16, tag=f"pdram{h}")
                nc.scalar.dma_start(
                    out=pd[:].rearrange("(d p) l -> p d l", p=128),
                    in_=partial_sb[:, off:off + ndt, :],
                )
                rs = dpool.tile([ndt * 16, 512], bf16, tag=f"rsout{h}")
                nc.gpsimd.collective_compute(
                    "ReduceScatter",
                    mybir.AluOpType.add,
                    replica_groups=[list(range(NCORES))],
                    ins=[pd.opt()],
                    outs=[rs.opt()],
                )
                rs_tiles.append(rs)
                off += ndt
            # SWDGE cast DMAs bf16 -> fp32 into the output
            r0 = 0
            for h, rs in enumerate(rs_tiles):
                n = RS_SPLIT[h] * 16
                nc.gpsimd.dma_start(
                    out=out_d.ap()[r0:r0 + n, :], in_=rs[:]
                )
                r0 += n

    nc.compile()
    return nc


def _host_prep(text, W, b):
    # tokens: [T, B] -> [B, T] int32, append bias + filler slots
    tok = np.ascontiguousarray(text.T).astype(np.int32)          # [B, T]
    extra = np.empty((B, 2), np.int32)
    extra[:, 0] = BIAS_SLOT
    extra[:, 1] = FILL_SLOT
    tok = np.concatenate([tok, extra], axis=1)                   # [B, TS]
    # partition-major pack: row p = docs {dt*128+p for dt in range(DT)}
    tok_pm = np.ascontiguousarray(
        tok.reshape(DT, 128, TS).transpose(1, 0, 2)
    ).reshape(128, DT * TS)

    chunk_base = np.repeat(np.arange(NCH, dtype=np.int32) * CH, TS)  # [NCH*TS]

    # weights: Wt [VP, 512] bf16, PAD column zeroed, bias row appended
    Wt = np.zeros((VP, L), np.float32)
    Wt[:V] = W.T
    Wt[PAD] = 0.0
    Wt[BIAS_SLOT] = b
    Wt = Wt.astype(ml_dtypes.bfloat16)

    in_maps = []
    for g in range(NCORES):
        bases_row = chunk_base + np.int32(g * VC)
        bases = np.broadcast_to(bases_row, (128, NCH * TS))
        tok_g = np.concatenate([tok_pm, bases], axis=1)          # [128, TOKW]
        wt_g = np.ascontiguousarray(
            Wt[g * VC:(g + 1) * VC].reshape(KT, 128, L).transpose(1, 0, 2)
        )                                                        # [128, KT, 512]
        in_maps.append({"tok": tok_g, "wt": wt_g})
    return in_maps


def kernel(text, W, b, trace=False, trace_kwargs=None):
    if "nc" not in _cache:
        _cache["nc"] = _build_nc()
    nc = _cache["nc"]
    in_maps = _host_prep(np.asarray(text), np.asarray(W), np.asarray(b))
    res = bass_utils.run_bass_kernel_spmd(
        nc, in_maps, core_ids=list(range(NCORES)),
        trace=trace, **(trace_kwargs or {}),
    )
    _cache["last_results"] = res
    out = np.empty((B, L), np.float32)
    for g in range(NCORES):
        og = res.results[g]["out"]
        r0, d0 = 0, 0
        for ndt in RS_SPLIT:
            n = ndt * 16
            out[d0 + n * g:d0 + n * g + n] = og[r0:r0 + n]
            r0 += n
            d0 += ndt * 128
    return out


# revision 16
# speedup vs baseline: 1.0356x; 1.0108x over previous
"""BOW multi-hot regression kernel for trn2, 8 NeuronCores.

score[b, l] = sum_{v in distinct non-PAD tokens of doc b} W[l, v] + bias[l]

Strategy (V-sharded, single SPMD launch):
  - vocab padded to 50176 = 8 * 6272; core g owns rows [6272g, 6272(g+1)).
  - W.T is host-prepped to bf16, PAD column zeroed, bias appended as row 50175
    which is activated through two constant extra token slots per doc.
  - per core: DVE computes per-chunk masked int16 indices; GPSIMD local_scatter
    builds the multi-hot bow [128 docs, 6272] bf16 per doc-tile (duplicate
    tokens overwrite the same cell with the same 1.0 -> dedup for free);
    HWDGE xbar DMA transposes it to ktile layout [128 v, 49, 128 docs];
    PE accumulates 49 matmuls per doc-tile into PSUM -> partial [1024, 512];
    one bf16 ReduceScatter sums partials; core g outputs docs [128g, 128(g+1)).
"""

import sys

sys.path.insert(0, "/opt/trn_rl_repo")

import numpy as np
import ml_dtypes

from concourse import bass, bacc, tile, mybir, bass_utils
from concourse.tile import add_dep_helper

# problem constants
T, B, V, L = 200, 1024, 50000, 512
PAD = 1
NCORES = 8
VP = 50176            # padded vocab, = NCORES * VC
VC = VP // NCORES     # 6272 vocab rows per core = KT * 128
KT = VC // 128        # 49 ktiles
CH = 1568             # local_scatter chunk width (1568 * 32 < 2**16)
NCH = VC // CH        # 4 chunks
TS = T + 2            # token slots per doc: 200 real + bias slot + filler
DT = B // 128         # 8 doc-tiles
BIAS_SLOT = VP - 1    # 50175
FILL_SLOT = VP - 2    # 50174
TOKW = DT * TS + NCH * TS  # flat tok row: DT doc slices then NCH chunk bases
RS_SPLIT = (6, 2)         # doc-tiles per ReduceScatter chunk

_cache = {}


def _build_nc():
    nc = bacc.Bacc("TRN2", target_bir_lowering=False, debug=False,
                   num_devices=NCORES)
    f32 = mybir.dt.float32
    bf16 = mybir.dt.bfloat16
    i32 = mybir.dt.int32
    i16 = mybir.dt.int16
    Alu = mybir.AluOpType

    # tok row layout per partition p: [DT, TS] tokens of docs {dt*128+p},
    # then [NCH, TS] chunk bases (core-local, host-filled)
    tok_d = nc.dram_tensor("tok", [128, TOKW], i32, kind="ExternalInput")
    wt_d = nc.dram_tensor("wt", [128, KT, 512], bf16, kind="ExternalInput")
    out_d = nc.dram_tensor("out", [128, 512], f32, kind="ExternalOutput")

    W808 = NCH * TS  # 808

    with tile.TileContext(nc) as tc:
        with tc.tile_pool(name="const", bufs=1) as cpool, \
             tc.tile_pool(name="work", bufs=3) as wpool, \
             tc.tile_pool(name="bow", bufs=4) as bpool, \
             tc.tile_pool(name="bowt", bufs=4) as btpool, \
             tc.tile_pool(name="psum", bufs=8, space="PSUM") as ppool, \
             tc.tile_pool(name="dram", bufs=1, space="DRAM") as dpool:

            tok_sb = cpool.tile([128, TOKW], i32, tag="tok")
            tok_dma = nc.sync.dma_start(out=tok_sb[:], in_=tok_d.ap())

            # wt: first ktiles immediately (needed by the first matmuls),
            # bulk deferred behind tok so tok wins the HBM bandwidth race
            wt_sb = cpool.tile([128, KT, 512], bf16, tag="wt")
            nc.scalar.dma_start(out=wt_sb[:, :8, :], in_=wt_d.ap()[:, :8, :])
            wt_bulk = nc.scalar.dma_start(
                out=wt_sb[:, 8:, :], in_=wt_d.ap()[:, 8:, :]
            )
            add_dep_helper(wt_bulk.ins, tok_dma.ins, sync=True,
                           reason="tok DMA gates the whole pipeline")

            ones_sb = cpool.tile([128, TS], bf16, tag="ones")
            nc.vector.memset(ones_sb[:], 1.0)

            # dummy scatter: loads the Q7 local_scatter library (~6us IRAM
            # DMA) during the preamble instead of on the critical path
            negi = cpool.tile([128, 2], i16, tag="negi")
            nc.vector.memset(negi[:], -1)
            scr = cpool.tile([128, 2], bf16, tag="scr")
            nc.gpsimd.local_scatter(
                scr[:], scr[:], negi[:], channels=128, num_elems=2, num_idxs=2,
            )

            partial_sb = cpool.tile([128, DT, 512], bf16, tag="partial")
            bases = tok_sb[:, DT * TS:].rearrange("p (c w) -> p c w", c=NCH)

            for dt in range(DT):
                # masked local chunk indices: for chunk c,
                # idx = tok - core_base - 1568*c  if in [0, 1568) else negative
                tokrep = (
                    tok_sb[:, dt * TS:(dt + 1) * TS]
                    .unsqueeze(1)
                    .broadcast_to((128, NCH, TS))
                )
                d_t = wpool.tile([128, NCH, TS], i32, tag="d")
                nc.vector.tensor_tensor(
                    out=d_t[:], in0=tokrep, in1=bases, op=Alu.subtract,
                )
                nc.vector.tensor_scalar(
                    out=d_t[:], in0=d_t[:],
                    scalar1=32767, scalar2=-1, op0=Alu.min, op1=Alu.max,
                )
                m_t = wpool.tile([128, NCH, TS], i32, tag="m")
                nc.vector.tensor_scalar(
                    out=m_t[:], in0=d_t[:],
                    scalar1=CH, scalar2=-32768, op0=Alu.is_ge, op1=Alu.mult,
                )
                idx_t = wpool.tile([128, NCH, TS], i16, tag="idx")
                nc.vector.tensor_tensor(
                    out=idx_t[:], in0=d_t[:], in1=m_t[:], op=Alu.add,
                )

                bow_t = bpool.tile([128, VC], bf16, tag="bow")
                for c in range(NCH):
                    nc.gpsimd.local_scatter(
                        bow_t[:, c * CH:(c + 1) * CH],
                        ones_sb[:],
                        idx_t[:, c, :],
                        channels=128,
                        num_elems=CH,
                        num_idxs=TS,
                    )

                bowt_t = btpool.tile([128, KT, 128], bf16, tag="bowt")
                ps = ppool.tile([128, 512], f32, tag="ps")
                k0 = 0
                for c in range(NCH):
                    k1 = ((c + 1) * CH) // 128 if c < NCH - 1 else KT
                    nc.sync.dma_start(
                        out=bowt_t[:, k0:k1, :],
                        in_=bow_t[:, k0 * 128:k1 * 128],
                        transpose=True,
                    )
                    for k in range(k0, k1):
                        nc.tensor.matmul(
                            out=ps[:],
                            lhsT=bowt_t[:, k, :],
                            rhs=wt_sb[:, k, :],
                            start=(k == 0),
                            stop=(k == KT - 1),
                        )
                    k0 = k1
                nc.vector.tensor_copy(out=partial_sb[:, dt, :], in_=ps[:])

            # chunked bf16 ReduceScatter: first chunk (dt 0..5) overlaps
            # the trailing matmuls; second chunk (dt 6..7) is the only
            # serial tail. Core g receives docs [96g, 96g+96) from chunk 0
            # and [768+32g, 768+32g+32) from chunk 1 (host reassembles).
            rs_tiles = []
            off = 0
            for h, ndt in enumerate(RS_SPLIT):
                pd = dpool.tile([ndt * 128, 512], bf16, tag=f"pdram{h}")
                nc.scalar.dma_start(
                    out=pd[:].rearrange("(d p) l -> p d l", p=128),
                    in_=partial_sb[:, off:off + ndt, :],
                )
                rs = dpool.tile([ndt * 16, 512], bf16, tag=f"rsout{h}")
                nc.gpsimd.collective_compute(
                    "ReduceScatter",
                    mybir.AluOpType.add,
                    replica_groups=[list(range(NCORES))],
                    ins=[pd.opt()],
                    outs=[rs.opt()],
                )
                rs_tiles.append(rs)
                off += ndt
            # SWDGE cast DMAs bf16 -> fp32 into the output
            r0 = 0
            for h, rs in enumerate(rs_tiles):
                n = RS_SPLIT[h] * 16
                nc.gpsimd.dma_start(
                    out=out_d.ap()[r0:r0 + n, :], in_=rs[:]
                )
                r0 += n

    nc.compile()
    return nc


def _host_prep(text, W, b):
    # tokens: [T, B] -> [B, T] int32, append bias + filler slots
    tok = np.ascontiguousarray(text.T).astype(np.int32)          # [B, T]
    extra = np.empty((B, 2), np.int32)
    extra[:, 0] = BIAS_SLOT
    extra[:, 1] = FILL_SLOT
    tok = np.concatenate([tok, extra], axis=1)                   # [B, TS]
    # partition-major pack: row p = docs {dt*128+p for dt in range(DT)}
    tok_pm = np.ascontiguousarray(
        tok.reshape(DT, 128, TS).transpose(1, 0, 2)
    ).reshape(128, DT * TS)

    chunk_base = np.repeat(np.arange(NCH, dtype=np.int32) * CH, TS)  # [NCH*TS]

    # weights: Wt [VP, 512] bf16, PAD column zeroed, bias row appended
    Wt = np.zeros((VP, L), np.float32)
    Wt[:V] = W.T
    Wt[PAD] = 0.0
    Wt[BIAS_SLOT] = b
    Wt = Wt.astype(ml_dtypes.bfloat16)

    in_maps = []
    for g in range(NCORES):
        bases_row = chunk_base + np.int32(g * VC)
        bases = np.broadcast_to(bases_row, (128, NCH * TS))
        tok_g = np.concatenate([tok_pm, bases], axis=1)          # [128, TOKW]
        wt_g = np.ascontiguousarray(
            Wt[g * VC:(g + 1) * VC].reshape(KT, 128, L).transpose(1, 0, 2)
        )                                                        # [128, KT, 512]
        in_maps.append({"tok": tok_g, "wt": wt_g})
    return in_maps


def kernel(text, W, b, trace=False, trace_kwargs=None):
    if "nc" not in _cache:
        _cache["nc"] = _build_nc()
    nc = _cache["nc"]
    in_maps = _host_prep(np.asarray(text), np.asarray(W), np.asarray(b))
    res = bass_utils.run_bass_kernel_spmd(
        nc, in_maps, core_ids=list(range(NCORES)),
        trace=trace, **(trace_kwargs or {}),
    )
    _cache["last_results"] = res
    out = np.empty((B, L), np.float32)
    for g in range(NCORES):
        og = res.results[g]["out"]
        r0, d0 = 0, 0
        for ndt in RS_SPLIT:
            n = ndt * 16
            out[d0 + n * g:d0 + n * g + n] = og[r0:r0 + n]
            r0 += n
            d0 += ndt * 128
    return out
